# revision 1
# baseline (speedup 1.0000x reference)
"""MultiHeadAttention Trainium2 kernel.

Full shapes: B=4, T=2048, D=1024, H=16, HD=64.
Sharding over 8 cores: core c -> batch b=c//2, head-group g=c%2 (8 heads,
feature columns g*512:(g+1)*512 of the projection space).

Per-core program (single SPMD Bass program, different input shards):
  - QKV projections in bf16 with fp32 PSUM accumulation (qp^T/kp^T kept
    [feat, tok]; vp kept [tok, feat]).
  - Attention per head-pair with scores TRANSPOSED ([tk, tq]) so that PV
    contracts tk on partitions; exp on ScalarE (scale=1/8 folded, no max
    subtraction -- scores are O(5) so fp32/bf16 exp is safe); softmax
    denominators via ones-column matmuls accumulated in PSUM; normalization
    via reciprocal_approx_fast + gpsimd partition_broadcast (broadcast only
    ever reads a base-partition-0 row / writes a full tile: sliced bases
    silently fail or hang).
  - Output projection vs Wo^T slice -> per-core partial out^T [1024, 2048];
    host sums the two head-group partials per batch and transposes.
"""

import os

import numpy as np

B, T, D, H = 4, 2048, 1024, 16
HD = 64
NCORES = 8
F = 512          # per-core projection features (8 heads * 64)
P = 128          # partitions
KT = D // P      # 8 k-tiles over D
MT = F // P      # 4 m-tiles over F (also head-pairs)
NCH = 4          # token chunks
CH = T // NCH    # 512 tokens per chunk
TKT = T // P     # 16 tk tiles
NH = F // HD     # 8 local heads

_CACHE = {}


def _build():
    import concourse.bass as bass
    import concourse.tile as tile
    from concourse import bacc, mybir
    from concourse.bass import ts

    f32 = mybir.dt.float32
    bf16 = mybir.dt.bfloat16

    nc = bacc.Bacc("TRN2", target_bir_lowering=False, debug=False)

    qT = nc.dram_tensor("qT", [D, T], bf16, kind="ExternalInput")
    kT = nc.dram_tensor("kT", [D, T], bf16, kind="ExternalInput")
    vT = nc.dram_tensor("vT", [D, T], bf16, kind="ExternalInput")
    wqT = nc.dram_tensor("wqT", [D, F], bf16, kind="ExternalInput")
    wkT = nc.dram_tensor("wkT", [D, F], bf16, kind="ExternalInput")
    wvT = nc.dram_tensor("wvT", [D, F], bf16, kind="ExternalInput")
    woT = nc.dram_tensor("woT", [F, D], bf16, kind="ExternalInput")
    bqs = nc.dram_tensor("bqs", [F], f32, kind="ExternalInput")
    bks = nc.dram_tensor("bks", [F], f32, kind="ExternalInput")
    bvs = nc.dram_tensor("bvs", [F], f32, kind="ExternalInput")
    bos = nc.dram_tensor("bos", [D], f32, kind="ExternalInput")
    outT = nc.dram_tensor("outT", [D, T], f32, kind="ExternalOutput")

    def r(x):  # matmul operand passthrough (bf16)
        return x

    with tile.TileContext(nc) as tc:
        from contextlib import ExitStack

        with ExitStack() as ctx:
            psum = ctx.enter_context(tc.tile_pool(name="ps", bufs=1, space="PSUM"))
            const = ctx.enter_context(tc.tile_pool(name="const", bufs=1))
            persist = ctx.enter_context(tc.tile_pool(name="persist", bufs=1))

            # ---- constants ----
            wo_sb = const.tile([P, MT, D], bf16, name="wo_sb")
            nc.sync.dma_start(
                out=wo_sb, in_=woT[:].rearrange("(ko ki) f -> ki ko f", ki=P)
            )
            bq_sb = const.tile([P, MT], f32, name="bq_sb")
            nc.sync.dma_start(out=bq_sb, in_=bqs[:].rearrange("(m p) -> p m", p=P))
            bk_sb = const.tile([P, MT], f32, name="bk_sb")
            nc.sync.dma_start(out=bk_sb, in_=bks[:].rearrange("(m p) -> p m", p=P))
            bo_sb = const.tile([P, D // P], f32, name="bo_sb")
            nc.sync.dma_start(out=bo_sb, in_=bos[:].rearrange("(m p) -> p m", p=P))
            # bv broadcast across partitions (bias in [tok, feat] layout)
            bv_bc = const.tile([P, F], f32, name="bv_bc")
            bvs_ap = bvs[:]
            nc.sync.dma_start(
                out=bv_bc,
                in_=bass.AP(
                    tensor=bvs_ap.tensor, offset=bvs_ap.offset,
                    ap=[[0, P], *bvs_ap.ap],
                ),
            )
            ones_pv = const.tile([P, 1], bf16, name="ones_pv")
            nc.vector.memset(ones_pv, 1.0)
            # Touch const tiles on DVE so later fused evictions carry only a
            # single sync wait (walrus TensorScalarPtr has one wait slot).
            touch = const.tile([P, 4], f32, name="touch")
            nc.vector.tensor_copy(out=touch, in_=bq_sb)
            nc.vector.tensor_copy(out=touch, in_=bk_sb)
            nc.vector.tensor_copy(out=touch[:, 0:1], in_=bv_bc[:, 0:1])
            nc.vector.tensor_copy(out=touch[:, 0:1], in_=bo_sb[:, 0:1])

            # ---- persistent activations ----
            qpt = persist.tile([P, MT, T], bf16, name="qpt")   # qp^T  [feat, tok]
            kpt = persist.tile([P, MT, T], bf16, name="kpt")   # kp^T
            vp = persist.tile([P, TKT, F], bf16, name="vp")   # vp    [tok, feat]

            # ================= projections =================
            with tc.tile_pool(name="wqkv", bufs=1) as wpool, \
                 tc.tile_pool(name="raw", bufs=1) as rpool:
                wq_sb = wpool.tile([P, KT, F], bf16, name="wq_sb")
                nc.sync.dma_start(
                    out=wq_sb, in_=wqT[:].rearrange("(ko ki) f -> ki ko f", ki=P)
                )
                wk_sb = wpool.tile([P, KT, F], bf16, name="wk_sb")
                nc.sync.dma_start(
                    out=wk_sb, in_=wkT[:].rearrange("(ko ki) f -> ki ko f", ki=P)
                )
                wv_sb = wpool.tile([P, KT, F], bf16, name="wv_sb")
                nc.sync.dma_start(
                    out=wv_sb, in_=wvT[:].rearrange("(ko ki) f -> ki ko f", ki=P)
                )

                for c in range(NCH):
                    cs = ts(c, CH)
                    q_raw = rpool.tile([P, KT, CH], bf16, name="q_raw", tag="raw", bufs=3)
                    nc.sync.dma_start(
                        out=q_raw,
                        in_=qT[:].rearrange("(ko ki) t -> ki ko t", ki=P)[:, :, cs],
                    )
                    k_raw = rpool.tile([P, KT, CH], bf16, name="k_raw", tag="raw", bufs=3)
                    nc.sync.dma_start(
                        out=k_raw,
                        in_=kT[:].rearrange("(ko ki) t -> ki ko t", ki=P)[:, :, cs],
                    )
                    v_raw = rpool.tile([P, KT, CH], bf16, name="v_raw", tag="raw", bufs=3)
                    nc.sync.dma_start(
                        out=v_raw,
                        in_=vT[:].rearrange("(ko ki) t -> ki ko t", ki=P)[:, :, cs],
                    )

                    # Q and K projections: out^T tiles [feat 128, tok 512]
                    for m in range(MT):
                        pq = psum.tile([P, CH], f32, name="pq", tag="mm", bufs=2)
                        for k in range(KT):
                            nc.tensor.matmul(
                                pq, lhsT=r(wq_sb[:, k, ts(m, P)]), rhs=r(q_raw[:, k, :]),
                                start=(k == 0), stop=(k == KT - 1),
                            )
                        nc.vector.tensor_scalar_add(
                            out=qpt[:, m, cs], in0=pq, scalar1=bq_sb[:, m : m + 1]
                        )
                        pk = psum.tile([P, CH], f32, name="pk", tag="mm", bufs=2)
                        for k in range(KT):
                            nc.tensor.matmul(
                                pk, lhsT=r(wk_sb[:, k, ts(m, P)]), rhs=r(k_raw[:, k, :]),
                                start=(k == 0), stop=(k == KT - 1),
                            )
                        nc.vector.tensor_scalar_add(
                            out=kpt[:, m, cs], in0=pk, scalar1=bk_sb[:, m : m + 1]
                        )
                    # V projection: natural layout tiles [tok 128, feat 512]
                    for tt in range(NCH):
                        pv_ = psum.tile([P, F], f32, name="pv_", tag="mm", bufs=2)
                        for k in range(KT):
                            nc.tensor.matmul(
                                pv_, lhsT=r(v_raw[:, k, ts(tt, P)]), rhs=r(wv_sb[:, k, :]),
                                start=(k == 0), stop=(k == KT - 1),
                            )
                        nc.vector.tensor_add(
                            out=vp[:, c * NCH + tt, :], in0=pv_, in1=bv_bc
                        )

            # ================= attention + output projection =================
            with tc.tile_pool(name="attn", bufs=1) as ap:
                for c in range(NCH):
                    cs = ts(c, CH)
                    aot = ap.tile([P, MT, CH], bf16, name="aot", tag="aot", bufs=2)
                    for p in range(MT):  # head pair (2p, 2p+1)
                        exp_h = ap.tile([P, TKT, CH], bf16, name="exp_h", tag="exp", bufs=3)
                        exp_h2 = ap.tile([P, TKT, CH], bf16, name="exp_h2", tag="exp", bufs=3)
                        pvp = psum.tile([P, CH], f32, name="pvp", tag="pv", bufs=1)
                        den = psum.tile([P, CH], f32, name="den", tag="den", bufs=1)
                        den2 = psum.tile([P, CH], f32, name="den2", tag="mm", bufs=2)
                        for jj in range(TKT // 2):  # tk tiles in groups of 2
                            qk_h = psum.tile([P, 2, CH], f32, name="qk_h", tag="qk", bufs=2)
                            qk_h2 = psum.tile([P, 2, CH], f32, name="qk_h2", tag="qk", bufs=2)
                            for u in range(2):
                                j = 2 * jj + u
                                # scores^T [tk, tq] for both heads (row-packed K=64)
                                nc.tensor.matmul(
                                    qk_h[:, u, :],
                                    lhsT=r(kpt[0:HD, p, ts(j, P)]),
                                    rhs=r(qpt[0:HD, p, cs]),
                                )
                                nc.tensor.matmul(
                                    qk_h2[:, u, :],
                                    lhsT=r(kpt[HD:P, p, ts(j, P)]),
                                    rhs=r(qpt[HD:P, p, cs]),
                                )
                            nc.scalar.activation(
                                out=exp_h[:, 2 * jj : 2 * jj + 2, :], in_=qk_h,
                                func=mybir.ActivationFunctionType.Exp, scale=0.125,
                            )
                            nc.scalar.activation(
                                out=exp_h2[:, 2 * jj : 2 * jj + 2, :], in_=qk_h2,
                                func=mybir.ActivationFunctionType.Exp, scale=0.125,
                            )
                        for j in range(TKT):
                            st, sp = (j == 0), (j == TKT - 1)
                            # PV: col-packed pair -> one PSUM bank rows 0-63 / 64-127
                            nc.tensor.matmul(
                                pvp[0:HD, :], lhsT=vp[:, j, ts(2 * p, HD)],
                                rhs=exp_h[:, j, :], start=st, stop=sp, skip_group_check=True,
                                tile_position=(0, 0),
                            )
                            nc.tensor.matmul(
                                pvp[HD:P, :], lhsT=vp[:, j, ts(2 * p + 1, HD)],
                                rhs=exp_h2[:, j, :], start=st, stop=sp, skip_group_check=True,
                                tile_position=(0, HD),
                            )
                            # softmax denominators (rows 0 and 32 of den bank)
                            nc.tensor.matmul(
                                den[0:1, :], lhsT=ones_pv, rhs=exp_h[:, j, :],
                                start=st, stop=sp, skip_group_check=True, tile_position=(0, 0),
                            )
                            nc.tensor.matmul(
                                den2[0:1, :], lhsT=ones_pv, rhs=exp_h2[:, j, :],
                                start=st, stop=sp, skip_group_check=True, tile_position=(0, 0),
                            )
                        rec = ap.tile([P, CH], f32, name="rec", tag="rec", bufs=2)
                        nc.vector.reciprocal(out=rec[0:1, :], in_=den[0:1, :])
                        rec2 = ap.tile([P, CH], f32, name="rec2", tag="rec2", bufs=2)
                        nc.vector.reciprocal(out=rec2[0:1, :], in_=den2[0:1, :])
                        bc1 = ap.tile([P, CH], f32, name="bc1", tag="bc1", bufs=2)
                        bc2 = ap.tile([P, CH], f32, name="bc2", tag="bc2", bufs=2)
                        nc.gpsimd.partition_broadcast(bc1[:, :], rec[0:1, :])
                        nc.gpsimd.partition_broadcast(bc2[:, :], rec2[0:1, :])
                        nc.vector.tensor_mul(
                            out=aot[0:HD, p, :], in0=pvp[0:HD, :], in1=bc1[0:HD, :]
                        )
                        nc.vector.tensor_mul(
                            out=aot[HD:P, p, :], in0=pvp[HD:P, :], in1=bc2[HD:P, :]
                        )
                    # output projection for this token chunk
                    for m in range(D // P):
                        po = psum.tile([P, CH], f32, name="po", tag="mm", bufs=2)
                        for p in range(MT):
                            nc.tensor.matmul(
                                po, lhsT=r(wo_sb[:, p, ts(m, P)]), rhs=r(aot[:, p, :]),
                                start=(p == 0), stop=(p == MT - 1),
                            )
                        ot = ap.tile([P, CH], f32, name="ot", tag="ot", bufs=3)
                        nc.vector.tensor_scalar_add(
                            out=ot, in0=po, scalar1=bo_sb[:, m : m + 1]
                        )
                        nc.sync.dma_start(out=outT[ts(m, P), cs], in_=ot)
    nc.compile()
    return nc


def kernel(q, k, v, Wq, bq, Wk, bk, Wv, bv, Wo, bo):
    from concourse.bass_utils import run_bass_kernel_spmd

    if "nc" not in _CACHE:
        _CACHE["nc"] = _build()
    nc = _CACHE["nc"]

    q, k, v = (np.asarray(x, np.float32) for x in (q, k, v))
    Wq, Wk, Wv, Wo = (np.asarray(x, np.float32) for x in (Wq, Wk, Wv, Wo))
    bq, bk, bv, bo = (np.asarray(x, np.float32) for x in (bq, bk, bv, bo))

    import ml_dtypes

    bf = ml_dtypes.bfloat16
    in_maps = []
    for c in range(NCORES):
        b, g = c // 2, c % 2
        cols = slice(g * F, (g + 1) * F)
        in_maps.append({
            "qT": np.ascontiguousarray(q[b].T).astype(bf),
            "kT": np.ascontiguousarray(k[b].T).astype(bf),
            "vT": np.ascontiguousarray(v[b].T).astype(bf),
            "wqT": np.ascontiguousarray(Wq[cols, :].T).astype(bf),
            "wkT": np.ascontiguousarray(Wk[cols, :].T).astype(bf),
            "wvT": np.ascontiguousarray(Wv[cols, :].T).astype(bf),
            "woT": np.ascontiguousarray(Wo[:, cols].T).astype(bf),
            "bqs": np.ascontiguousarray(bq[cols]),
            "bks": np.ascontiguousarray(bk[cols]),
            "bvs": np.ascontiguousarray(bv[cols]),
            # bo applied once per batch (head-group 0 only)
            "bos": np.ascontiguousarray(bo if g == 0 else np.zeros_like(bo)),
        })

    _CACHE["in_maps"] = in_maps
    trace = bool(int(os.environ.get("KERNEL_TRACE", "0")))
    res = run_bass_kernel_spmd(
        nc, in_maps, core_ids=list(range(NCORES)), trace=trace
    )
    if trace and res.exec_time_ns is not None:
        print(f"HW exec time: {res.exec_time_ns} ns")
    outs = [r["outT"] for r in res.results]
    out = np.empty((B, T, D), np.float32)
    for b in range(B):
        out[b] = (outs[2 * b] + outs[2 * b + 1]).T
    return out



# revision 3
# speedup vs baseline: 1.6688x; 1.6688x over previous
"""MultiHeadAttention Trainium2 kernel.

Full shapes: B=4, T=2048, D=1024, H=16, HD=64.
Sharding over 8 cores: core c -> batch b=c//2, head-group g=c%2 (8 heads,
feature columns g*512:(g+1)*512 of the projection space).

Per-core program (single SPMD Bass program, different input shards):
  - K/V projections first (bf16, fp32 PSUM); Q projection folded into the
    attention chunk loop so it fills TensorE idle while ScalarE exps.
  - V is stored as vp4[tok, tkt, head, 65]: 64 projected features plus a
    ones column, so each PV matmul (M=65) accumulates the softmax
    denominator in PSUM row 64 for free -- no separate ones-matmuls.
  - Scores TRANSPOSED ([tk, tq]); two heads row-packed on the PE via
    base-partition 0/64 (auto tile_position), two j-tiles per head into one
    [128, 4, 512] PSUM tile so a single ScalarE Exp covers 2048 elem/part.
  - Tail per head: reciprocal_approx_fast (den row 64 -> partition 0;
    1-partition DVE ops may cross quadrants), gpsimd partition_broadcast,
    DVE mul into aot (64-partition writes to rows 64-127 read rows 0-63).
  - Output projection vs Wo^T slice -> per-core partial out^T [1024, 2048];
    host sums the two head-group partials per batch and transposes.
"""

import os

import numpy as np

B, T, D, H = 4, 2048, 1024, 16
HD = 64
NCORES = 8
F = 512          # per-core projection features (8 heads * 64)
P = 128          # partitions
KT = D // P      # 8 k-tiles over D
MT = F // P      # 4 m-tiles over F (also head-pairs)
NCH = 4          # token chunks
CH = T // NCH    # 512 tokens per chunk
TKT = T // P     # 16 tk tiles
NH = F // HD     # 8 local heads

_CACHE = {}


def _build():
    import concourse.bass as bass
    import concourse.tile as tile
    from concourse import bacc, mybir
    from concourse.bass import ts

    f32 = mybir.dt.float32
    bf16 = mybir.dt.bfloat16

    nc = bacc.Bacc("TRN2", target_bir_lowering=False, debug=False)

    qT = nc.dram_tensor("qT", [D, T], bf16, kind="ExternalInput")
    kT = nc.dram_tensor("kT", [D, T], bf16, kind="ExternalInput")
    vT = nc.dram_tensor("vT", [D, T], bf16, kind="ExternalInput")
    wqT = nc.dram_tensor("wqT", [D, F], bf16, kind="ExternalInput")
    wkT = nc.dram_tensor("wkT", [D, F], bf16, kind="ExternalInput")
    wvT = nc.dram_tensor("wvT", [D, F], bf16, kind="ExternalInput")
    woT = nc.dram_tensor("woT", [F, D], bf16, kind="ExternalInput")
    bqs = nc.dram_tensor("bqs", [F], f32, kind="ExternalInput")
    bks = nc.dram_tensor("bks", [F], f32, kind="ExternalInput")
    bvs = nc.dram_tensor("bvs", [F], f32, kind="ExternalInput")
    bos = nc.dram_tensor("bos", [D], f32, kind="ExternalInput")
    outT = nc.dram_tensor("outT", [D, T], f32, kind="ExternalOutput")

    with tile.TileContext(nc) as tc:
        from contextlib import ExitStack

        with ExitStack() as ctx:
            psum = ctx.enter_context(tc.tile_pool(name="ps", bufs=1, space="PSUM"))
            const = ctx.enter_context(tc.tile_pool(name="const", bufs=1))
            persist = ctx.enter_context(tc.tile_pool(name="persist", bufs=1))
            ap = ctx.enter_context(tc.tile_pool(name="work", bufs=1))

            # ---- constants ----
            wq_sb = const.tile([P, KT, F], bf16, name="wq_sb")
            nc.sync.dma_start(
                out=wq_sb, in_=wqT[:].rearrange("(ko ki) f -> ki ko f", ki=P)
            )
            wk_sb = const.tile([P, KT, F], bf16, name="wk_sb")
            nc.sync.dma_start(
                out=wk_sb, in_=wkT[:].rearrange("(ko ki) f -> ki ko f", ki=P)
            )
            wv_sb = const.tile([P, KT, F], bf16, name="wv_sb")
            nc.sync.dma_start(
                out=wv_sb, in_=wvT[:].rearrange("(ko ki) f -> ki ko f", ki=P)
            )
            wo_sb = const.tile([P, MT, D], bf16, name="wo_sb")
            nc.sync.dma_start(
                out=wo_sb, in_=woT[:].rearrange("(ko ki) f -> ki ko f", ki=P)
            )
            bq_sb = const.tile([P, MT], f32, name="bq_sb")
            nc.sync.dma_start(out=bq_sb, in_=bqs[:].rearrange("(m p) -> p m", p=P))
            bk_sb = const.tile([P, MT], f32, name="bk_sb")
            nc.sync.dma_start(out=bk_sb, in_=bks[:].rearrange("(m p) -> p m", p=P))
            bo_sb = const.tile([P, D // P], f32, name="bo_sb")
            nc.sync.dma_start(out=bo_sb, in_=bos[:].rearrange("(m p) -> p m", p=P))
            # bv broadcast across partitions (bias in [tok, feat] layout)
            bv_bc = const.tile([P, F], f32, name="bv_bc")
            bvs_ap = bvs[:]
            nc.sync.dma_start(
                out=bv_bc,
                in_=bass.AP(
                    tensor=bvs_ap.tensor, offset=bvs_ap.offset,
                    ap=[[0, P], *bvs_ap.ap],
                ),
            )
            # Touch const tiles on DVE so later fused evictions carry only a
            # single sync wait (walrus TensorScalarPtr has one wait slot).
            touch = const.tile([P, 4], f32, name="touch")
            nc.vector.tensor_copy(out=touch, in_=bq_sb)
            nc.vector.tensor_copy(out=touch, in_=bk_sb)
            nc.vector.tensor_copy(out=touch[:, 0:1], in_=bv_bc[:, 0:1])
            nc.vector.tensor_copy(out=touch[:, 0:1], in_=bo_sb[:, 0:1])

            # ---- persistent activations ----
            qpt = persist.tile([P, MT, T], bf16, name="qpt")   # qp^T  [feat, tok]
            kpt = persist.tile([P, MT, T], bf16, name="kpt")   # kp^T
            # vp4: [tok, tkt, head, HD+1]; col HD is the ones column that
            # makes each PV matmul emit the softmax denominator in row HD.
            vp4 = persist.tile([P, TKT, NH, HD + 1], bf16, name="vp4")
            nc.vector.memset(vp4[:, :, :, HD : HD + 1], 1.0)

            # ================= K/V projections =================
            for c in range(NCH):
                cs = ts(c, CH)
                k_raw = ap.tile([P, KT, CH], bf16, name="k_raw", tag="raw", bufs=3)
                nc.sync.dma_start(
                    out=k_raw,
                    in_=kT[:].rearrange("(ko ki) t -> ki ko t", ki=P)[:, :, cs],
                )
                v_raw = ap.tile([P, KT, CH], bf16, name="v_raw", tag="raw", bufs=3)
                nc.sync.dma_start(
                    out=v_raw,
                    in_=vT[:].rearrange("(ko ki) t -> ki ko t", ki=P)[:, :, cs],
                )
                for m in range(MT):
                    pk = psum.tile([P, CH], f32, name="pk", tag="mm", bufs=2)
                    for k in range(KT):
                        nc.tensor.matmul(
                            pk, lhsT=wk_sb[:, k, ts(m, P)], rhs=k_raw[:, k, :],
                            start=(k == 0), stop=(k == KT - 1),
                        )
                    nc.vector.tensor_scalar_add(
                        out=kpt[:, m, cs], in0=pk, scalar1=bk_sb[:, m : m + 1]
                    )
                for tt in range(NCH):
                    pv_ = psum.tile([P, F], f32, name="pv_", tag="mm", bufs=2)
                    for k in range(KT):
                        nc.tensor.matmul(
                            pv_, lhsT=v_raw[:, k, ts(tt, P)], rhs=wv_sb[:, k, :],
                            start=(k == 0), stop=(k == KT - 1),
                        )
                    nc.vector.tensor_add(
                        out=vp4[:, c * NCH + tt, :, 0:HD], in0=pv_, in1=bv_bc
                    )

            # ============ attention (+Q proj, +output proj) ============
            for c in range(NCH):
                cs = ts(c, CH)
                # Q projection for this chunk
                q_raw = ap.tile([P, KT, CH], bf16, name="q_raw", tag="raw", bufs=3)
                nc.sync.dma_start(
                    out=q_raw,
                    in_=qT[:].rearrange("(ko ki) t -> ki ko t", ki=P)[:, :, cs],
                )
                for m in range(MT):
                    pq = psum.tile([P, CH], f32, name="pq", tag="mm", bufs=2)
                    for k in range(KT):
                        nc.tensor.matmul(
                            pq, lhsT=wq_sb[:, k, ts(m, P)], rhs=q_raw[:, k, :],
                            start=(k == 0), stop=(k == KT - 1),
                        )
                    nc.vector.tensor_scalar_add(
                        out=qpt[:, m, cs], in0=pq, scalar1=bq_sb[:, m : m + 1]
                    )

                aot = ap.tile([P, MT, CH], bf16, name="aot", tag="aot", bufs=2)
                for p in range(MT):  # head pair (2p, 2p+1)
                    hA, hB = 2 * p, 2 * p + 1
                    pvA = psum.tile([P, CH], f32, name="pvA", tag="pv", bufs=2)
                    pvB = psum.tile([P, CH], f32, name="pvB", tag="pv", bufs=2)
                    for j in range(TKT):
                        qk = psum.tile([P, 2, CH], f32, name="qk", tag="qk", bufs=2)
                        # scores^T [tk, tq]; heads row-packed at rows 0/64
                        nc.tensor.matmul(
                            qk[:, 0, :], lhsT=kpt[0:HD, p, ts(j, P)],
                            rhs=qpt[0:HD, p, cs],
                        )
                        nc.tensor.matmul(
                            qk[:, 1, :], lhsT=kpt[HD:P, p, ts(j, P)],
                            rhs=qpt[HD:P, p, cs],
                        )
                        exph = ap.tile([P, 2, CH], bf16, name="exph", tag="exp", bufs=4)
                        nc.scalar.activation(
                            out=exph, in_=qk,
                            func=mybir.ActivationFunctionType.Exp, scale=0.125,
                        )
                        # PV accumulation (M=65: row 64 = denominator)
                        nc.tensor.matmul(
                            pvA[0:HD + 1, :], lhsT=vp4[:, j, hA, :],
                            rhs=exph[:, 0, :], start=(j == 0), stop=(j == TKT - 1),
                        )
                        nc.tensor.matmul(
                            pvB[0:HD + 1, :], lhsT=vp4[:, j, hB, :],
                            rhs=exph[:, 1, :], start=(j == 0), stop=(j == TKT - 1),
                        )
                    # normalization tail: evacuate den row 64 -> partition 0
                    # (plain copy may cross quadrants; reciprocal_approx_fast
                    # only works at base partition 0)
                    denA = ap.tile([P, CH], f32, name="denA", tag="den", bufs=2)
                    nc.vector.tensor_copy(out=denA[0:1, :], in_=pvA[HD : HD + 1, :])
                    denB = ap.tile([P, CH], f32, name="denB", tag="den", bufs=2)
                    nc.vector.tensor_copy(out=denB[0:1, :], in_=pvB[HD : HD + 1, :])
                    recA = ap.tile([P, CH], f32, name="recA", tag="rec", bufs=2)
                    nc.vector.reciprocal_approx_fast(
                        out=recA[0:1, :], in_=denA[0:1, :]
                    )
                    recB = ap.tile([P, CH], f32, name="recB", tag="rec", bufs=2)
                    nc.vector.reciprocal_approx_fast(
                        out=recB[0:1, :], in_=denB[0:1, :]
                    )
                    bcA = ap.tile([P, CH], f32, name="bcA", tag="bc", bufs=2)
                    nc.gpsimd.partition_broadcast(bcA[:, :], recA[0:1, :])
                    bcB = ap.tile([P, CH], f32, name="bcB", tag="bc", bufs=2)
                    nc.gpsimd.partition_broadcast(bcB[:, :], recB[0:1, :])
                    nc.vector.tensor_mul(
                        out=aot[0:HD, p, :], in0=pvA[0:HD, :], in1=bcA[0:HD, :]
                    )
                    nc.vector.tensor_mul(
                        out=aot[HD:P, p, :], in0=pvB[0:HD, :], in1=bcB[0:HD, :]
                    )
                # output projection for this token chunk
                for m in range(D // P):
                    po = psum.tile([P, CH], f32, name="po", tag="mm", bufs=2)
                    for p in range(MT):
                        nc.tensor.matmul(
                            po, lhsT=wo_sb[:, p, ts(m, P)], rhs=aot[:, p, :],
                            start=(p == 0), stop=(p == MT - 1),
                        )
                    ot = ap.tile([P, CH], f32, name="ot", tag="ot", bufs=3)
                    nc.vector.tensor_scalar_add(
                        out=ot, in0=po, scalar1=bo_sb[:, m : m + 1]
                    )
                    nc.sync.dma_start(out=outT[ts(m, P), cs], in_=ot)
    nc.compile()
    return nc


def kernel(q, k, v, Wq, bq, Wk, bk, Wv, bv, Wo, bo):
    from concourse.bass_utils import run_bass_kernel_spmd

    if "nc" not in _CACHE:
        _CACHE["nc"] = _build()
    nc = _CACHE["nc"]

    q, k, v = (np.asarray(x, np.float32) for x in (q, k, v))
    Wq, Wk, Wv, Wo = (np.asarray(x, np.float32) for x in (Wq, Wk, Wv, Wo))
    bq, bk, bv, bo = (np.asarray(x, np.float32) for x in (bq, bk, bv, bo))

    import ml_dtypes

    bf = ml_dtypes.bfloat16
    in_maps = []
    for c in range(NCORES):
        b, g = c // 2, c % 2
        cols = slice(g * F, (g + 1) * F)
        in_maps.append({
            "qT": np.ascontiguousarray(q[b].T).astype(bf),
            "kT": np.ascontiguousarray(k[b].T).astype(bf),
            "vT": np.ascontiguousarray(v[b].T).astype(bf),
            "wqT": np.ascontiguousarray(Wq[cols, :].T).astype(bf),
            "wkT": np.ascontiguousarray(Wk[cols, :].T).astype(bf),
            "wvT": np.ascontiguousarray(Wv[cols, :].T).astype(bf),
            "woT": np.ascontiguousarray(Wo[:, cols].T).astype(bf),
            "bqs": np.ascontiguousarray(bq[cols]),
            "bks": np.ascontiguousarray(bk[cols]),
            "bvs": np.ascontiguousarray(bv[cols]),
            # bo applied once per batch (head-group 0 only)
            "bos": np.ascontiguousarray(bo if g == 0 else np.zeros_like(bo)),
        })

    _CACHE["in_maps"] = in_maps
    trace = bool(int(os.environ.get("KERNEL_TRACE", "0")))
    res = run_bass_kernel_spmd(
        nc, in_maps, core_ids=list(range(NCORES)), trace=trace
    )
    if trace and res.exec_time_ns is not None:
        print(f"HW exec time: {res.exec_time_ns} ns")
    outs = [r["outT"] for r in res.results]
    out = np.empty((B, T, D), np.float32)
    for b in range(B):
        out[b] = (outs[2 * b] + outs[2 * b + 1]).T
    return out


# revision 4
# speedup vs baseline: 2.0919x; 1.2536x over previous
"""MultiHeadAttention Trainium2 kernel.

Full shapes: B=4, T=2048, D=1024, H=16, HD=64.
Sharding over 8 cores: core c -> batch b=c//2, head-group g=c%2 (8 heads,
feature columns g*512:(g+1)*512 of the projection space).

Per-core program (single SPMD Bass program, different input shards):
  - Phase 1: K/V projections (bf16, fp32 PSUM) + Q projection of chunk 0.
  - V stored as vp4[tok, tkt, head, 65]: 64 projected features plus a ones
    column, so each PV matmul (M=65) accumulates the softmax denominator in
    PSUM row 64 for free -- no separate ones-matmuls.
  - Phase 2: flat software-pipelined batch stream over (chunk, pair, j):
    scores^T [tk, tq] with two heads row-packed at PE rows 0/64; one
    ScalarE Exp per [128, 2, 512] PSUM tile (double-buffered). PV matmuls
    are emitted with a 3-batch lag so the per-pair normalization tail never
    head-of-line-blocks the PE queue. Q-proj of chunk c+1 and out-proj of
    chunk c-1 are interleaved as fillers into the ACT-bound stream.
  - Tail per head: DVE copy of den row 64 -> partition 0 (plain copies may
    cross quadrants; reciprocal_approx_fast works only at base 0), fast
    reciprocal, gpsimd partition_broadcast, DVE mul into aot.
  - Output projection vs Wo^T slice -> per-core partial out^T [1024, 2048];
    host sums the two head-group partials per batch and transposes.
"""

import os
from collections import deque

import numpy as np

B, T, D, H = 4, 2048, 1024, 16
HD = 64
NCORES = 8
F = 512          # per-core projection features (8 heads * 64)
P = 128          # partitions
KT = D // P      # 8 k-tiles over D
MT = F // P      # 4 m-tiles over F (also head-pairs)
NCH = 4          # token chunks
CH = T // NCH    # 512 tokens per chunk
TKT = T // P     # 16 tk tiles
NH = F // HD     # 8 local heads

_CACHE = {}


def _build():
    import concourse.bass as bass
    import concourse.tile as tile
    from concourse import bacc, mybir
    from concourse.bass import ts

    f32 = mybir.dt.float32
    bf16 = mybir.dt.bfloat16

    nc = bacc.Bacc("TRN2", target_bir_lowering=False, debug=False)

    qT = nc.dram_tensor("qT", [D, T], bf16, kind="ExternalInput")
    kT = nc.dram_tensor("kT", [D, T], bf16, kind="ExternalInput")
    vT = nc.dram_tensor("vT", [D, T], bf16, kind="ExternalInput")
    wqT = nc.dram_tensor("wqT", [D, F], bf16, kind="ExternalInput")
    wkT = nc.dram_tensor("wkT", [D, F], bf16, kind="ExternalInput")
    wvT = nc.dram_tensor("wvT", [D, F], bf16, kind="ExternalInput")
    woT = nc.dram_tensor("woT", [F, D], bf16, kind="ExternalInput")
    bqs = nc.dram_tensor("bqs", [F], f32, kind="ExternalInput")
    bks = nc.dram_tensor("bks", [F], f32, kind="ExternalInput")
    bvs = nc.dram_tensor("bvs", [F], f32, kind="ExternalInput")
    bos = nc.dram_tensor("bos", [D], f32, kind="ExternalInput")
    outT = nc.dram_tensor("outT", [D, T], f32, kind="ExternalOutput")

    with tile.TileContext(nc) as tc:
        from contextlib import ExitStack

        with ExitStack() as ctx:
            psum = ctx.enter_context(tc.tile_pool(name="ps", bufs=1, space="PSUM"))
            const = ctx.enter_context(tc.tile_pool(name="const", bufs=1))
            persist = ctx.enter_context(tc.tile_pool(name="persist", bufs=1))
            ap = ctx.enter_context(tc.tile_pool(name="work", bufs=1))

            def raw_load(src, c, nm):
                t = ap.tile([P, KT, CH], bf16, name=nm, tag="raw", bufs=3)
                nc.sync.dma_start(
                    out=t,
                    in_=src[:].rearrange("(ko ki) t -> ki ko t", ki=P)[:, :, ts(c, CH)],
                )
                return t

            # ---- constants: K/V path first so the first matmul starts early
            wk_sb = const.tile([P, KT, F], bf16, name="wk_sb")
            nc.sync.dma_start(
                out=wk_sb, in_=wkT[:].rearrange("(ko ki) f -> ki ko f", ki=P)
            )
            bk_sb = const.tile([P, MT], f32, name="bk_sb")
            nc.sync.dma_start(out=bk_sb, in_=bks[:].rearrange("(m p) -> p m", p=P))
            k_raw0 = raw_load(kT, 0, "k_raw")
            wv_sb = const.tile([P, KT, F], bf16, name="wv_sb")
            nc.sync.dma_start(
                out=wv_sb, in_=wvT[:].rearrange("(ko ki) f -> ki ko f", ki=P)
            )
            v_raw0 = raw_load(vT, 0, "v_raw")
            wq_sb = const.tile([P, KT, F], bf16, name="wq_sb")
            nc.sync.dma_start(
                out=wq_sb, in_=wqT[:].rearrange("(ko ki) f -> ki ko f", ki=P)
            )
            wo_sb = const.tile([P, MT, D], bf16, name="wo_sb")
            nc.sync.dma_start(
                out=wo_sb, in_=woT[:].rearrange("(ko ki) f -> ki ko f", ki=P)
            )
            bq_sb = const.tile([P, MT], f32, name="bq_sb")
            nc.sync.dma_start(out=bq_sb, in_=bqs[:].rearrange("(m p) -> p m", p=P))
            bo_sb = const.tile([P, D // P], f32, name="bo_sb")
            nc.sync.dma_start(out=bo_sb, in_=bos[:].rearrange("(m p) -> p m", p=P))
            # bv broadcast across partitions (bias in [tok, feat] layout)
            bv_bc = const.tile([P, F], f32, name="bv_bc")
            bvs_ap = bvs[:]
            nc.sync.dma_start(
                out=bv_bc,
                in_=bass.AP(
                    tensor=bvs_ap.tensor, offset=bvs_ap.offset,
                    ap=[[0, P], *bvs_ap.ap],
                ),
            )
            # Touch const tiles on DVE so later fused evictions carry only a
            # single sync wait (walrus TensorScalarPtr has one wait slot).
            touch = const.tile([P, 4], f32, name="touch")
            nc.vector.tensor_copy(out=touch, in_=bq_sb)
            nc.vector.tensor_copy(out=touch, in_=bk_sb)
            nc.vector.tensor_copy(out=touch[:, 0:1], in_=bv_bc[:, 0:1])
            nc.vector.tensor_copy(out=touch[:, 0:1], in_=bo_sb[:, 0:1])

            # ---- persistent activations ----
            qpt = persist.tile([P, MT, T], bf16, name="qpt")   # qp^T  [feat, tok]
            kpt = persist.tile([P, MT, T], bf16, name="kpt")   # kp^T
            # vp4: [tok, tkt, head, HD+1]; col HD = ones (softmax denominator)
            vp4 = persist.tile([P, TKT, NH, HD + 1], bf16, name="vp4")
            nc.vector.memset(vp4[:, :, :, HD : HD + 1], 1.0)

            def qproj_tile(c, m, q_raw):
                pq = psum.tile([P, CH], f32, name="pq", tag="mm", bufs=1)
                for k in range(KT):
                    nc.tensor.matmul(
                        pq, lhsT=wq_sb[:, k, ts(m, P)], rhs=q_raw[:, k, :],
                        start=(k == 0), stop=(k == KT - 1),
                    )
                nc.vector.tensor_scalar_add(
                    out=qpt[:, m, ts(c, CH)], in0=pq, scalar1=bq_sb[:, m : m + 1]
                )

            # ================= phase 1: K/V projections + Q0 ==============
            # (phase-1 projection PSUM rotates through the attention "pv"
            # tag, which is otherwise idle here)
            for c in range(NCH):
                k_raw = k_raw0 if c == 0 else raw_load(kT, c, "k_raw")
                v_raw = v_raw0 if c == 0 else raw_load(vT, c, "v_raw")
                for m in range(MT):
                    pk = psum.tile([P, CH], f32, name="pk", tag="pv", bufs=3)
                    for k in range(KT):
                        nc.tensor.matmul(
                            pk, lhsT=wk_sb[:, k, ts(m, P)], rhs=k_raw[:, k, :],
                            start=(k == 0), stop=(k == KT - 1),
                        )
                    nc.vector.tensor_scalar_add(
                        out=kpt[:, m, ts(c, CH)], in0=pk, scalar1=bk_sb[:, m : m + 1]
                    )
                for tt in range(NCH):
                    pv_ = psum.tile([P, F], f32, name="pv_", tag="pv", bufs=3)
                    for k in range(KT):
                        nc.tensor.matmul(
                            pv_, lhsT=v_raw[:, k, ts(tt, P)], rhs=wv_sb[:, k, :],
                            start=(k == 0), stop=(k == KT - 1),
                        )
                    nc.vector.tensor_add(
                        out=vp4[:, c * NCH + tt, :, 0:HD], in0=pv_, in1=bv_bc
                    )
            q_raw_c = raw_load(qT, 0, "q_raw")
            for m in range(MT):
                pq = psum.tile([P, CH], f32, name="pq0", tag="pv", bufs=3)
                for k in range(KT):
                    nc.tensor.matmul(
                        pq, lhsT=wq_sb[:, k, ts(m, P)], rhs=q_raw_c[:, k, :],
                        start=(k == 0), stop=(k == KT - 1),
                    )
                nc.vector.tensor_scalar_add(
                    out=qpt[:, m, ts(0, CH)], in0=pq, scalar1=bq_sb[:, m : m + 1]
                )

            # ============ phase 2: pipelined attention stream =============
            LAG = 3
            SP = 5  # one filler every SP batches
            pvq = deque()      # pending PV emissions: (c, p, j, exph)
            fillers = deque()  # deferred PE work (q-proj / out-proj m-tiles)
            pair_tiles = {}    # (c, p) -> (pvA, pvB)
            aot_tiles = {}     # c -> aot

            def emit_tail(c, p):
                pvA, pvB = pair_tiles.pop((c, p))
                aot = aot_tiles.get(c)
                if aot is None:
                    aot = ap.tile([P, MT, CH], bf16, name="aot", tag="aot", bufs=2)
                    aot_tiles[c] = aot
                denA = ap.tile([P, CH], f32, name="denA", tag="den", bufs=2)
                nc.vector.tensor_copy(out=denA[0:1, :], in_=pvA[HD : HD + 1, :])
                recA = ap.tile([P, CH], f32, name="recA", tag="rec", bufs=2)
                nc.vector.reciprocal_approx_fast(out=recA[0:1, :], in_=denA[0:1, :])
                denB = ap.tile([P, CH], f32, name="denB", tag="den", bufs=2)
                nc.vector.tensor_copy(out=denB[0:1, :], in_=pvB[HD : HD + 1, :])
                recB = ap.tile([P, CH], f32, name="recB", tag="rec", bufs=2)
                nc.vector.reciprocal_approx_fast(out=recB[0:1, :], in_=denB[0:1, :])
                bcA = ap.tile([P, CH], f32, name="bcA", tag="bc", bufs=2)
                nc.gpsimd.partition_broadcast(bcA[:, :], recA[0:1, :])
                bcB = ap.tile([P, CH], f32, name="bcB", tag="bc", bufs=2)
                nc.gpsimd.partition_broadcast(bcB[:, :], recB[0:1, :])
                nc.vector.tensor_mul(
                    out=aot[0:HD, p, :], in0=pvA[0:HD, :], in1=bcA[0:HD, :]
                )
                nc.vector.tensor_mul(
                    out=aot[HD:P, p, :], in0=pvB[0:HD, :], in1=bcB[0:HD, :]
                )

            def emit_pv(c, p, j, exph):
                tiles = pair_tiles.get((c, p))
                if tiles is None:
                    pvA = psum.tile([P, CH], f32, name="pvA", tag="pv", bufs=3)
                    pvB = psum.tile([P, CH], f32, name="pvB", tag="pv", bufs=3)
                    tiles = pair_tiles[(c, p)] = (pvA, pvB)
                pvA, pvB = tiles
                st, sp_ = (j == 0), (j == TKT - 1)
                nc.tensor.matmul(
                    pvA[0:HD + 1, :], lhsT=vp4[:, j, 2 * p, :],
                    rhs=exph[:, 0, :], start=st, stop=sp_,
                )
                nc.tensor.matmul(
                    pvB[0:HD + 1, :], lhsT=vp4[:, j, 2 * p + 1, :],
                    rhs=exph[:, 1, :], start=st, stop=sp_,
                )
                if sp_:
                    emit_tail(c, p)

            def oproj_tile(c, m):
                aot = aot_tiles[c]
                po = psum.tile([P, CH], f32, name="po", tag="mm", bufs=1)
                for p in range(MT):
                    nc.tensor.matmul(
                        po, lhsT=wo_sb[:, p, ts(m, P)], rhs=aot[:, p, :],
                        start=(p == 0), stop=(p == MT - 1),
                    )
                ot = ap.tile([P, CH], f32, name="ot", tag="ot", bufs=3)
                nc.vector.tensor_scalar_add(
                    out=ot, in0=po, scalar1=bo_sb[:, m : m + 1]
                )
                nc.sync.dma_start(out=outT[ts(m, P), ts(c, CH)], in_=ot)

            bi = 0
            for c in range(NCH):
                if c + 1 < NCH:
                    q_raw_n = raw_load(qT, c + 1, "q_raw")
                    for m in range(MT):
                        fillers.append(
                            lambda c_=c + 1, m_=m, r_=q_raw_n: qproj_tile(c_, m_, r_)
                        )
                if c - 1 >= 0:
                    for m in range(D // P):
                        fillers.append(lambda c_=c - 1, m_=m: oproj_tile(c_, m_))
                cs = ts(c, CH)
                for p in range(MT):
                    for j in range(TKT):
                        qk = psum.tile([P, 2, CH], f32, name="qk", tag="qk", bufs=2)
                        nc.tensor.matmul(
                            qk[:, 0, :], lhsT=kpt[0:HD, p, ts(j, P)],
                            rhs=qpt[0:HD, p, cs],
                        )
                        nc.tensor.matmul(
                            qk[:, 1, :], lhsT=kpt[HD:P, p, ts(j, P)],
                            rhs=qpt[HD:P, p, cs],
                        )
                        exph = ap.tile([P, 2, CH], bf16, name="exph", tag="exp", bufs=6)
                        nc.scalar.activation(
                            out=exph, in_=qk,
                            func=mybir.ActivationFunctionType.Exp, scale=0.125,
                        )
                        pvq.append((c, p, j, exph))
                        if len(pvq) > LAG:
                            emit_pv(*pvq.popleft())
                        bi += 1
                        if bi % SP == 0 and fillers:
                            fillers.popleft()()
            while pvq:
                emit_pv(*pvq.popleft())
            while fillers:
                fillers.popleft()()
            for m in range(D // P):
                oproj_tile(NCH - 1, m)
    nc.compile()
    return nc


def kernel(q, k, v, Wq, bq, Wk, bk, Wv, bv, Wo, bo):
    from concourse.bass_utils import run_bass_kernel_spmd

    if "nc" not in _CACHE:
        _CACHE["nc"] = _build()
    nc = _CACHE["nc"]

    q, k, v = (np.asarray(x, np.float32) for x in (q, k, v))
    Wq, Wk, Wv, Wo = (np.asarray(x, np.float32) for x in (Wq, Wk, Wv, Wo))
    bq, bk, bv, bo = (np.asarray(x, np.float32) for x in (bq, bk, bv, bo))

    import ml_dtypes

    bf = ml_dtypes.bfloat16
    in_maps = []
    for c in range(NCORES):
        b, g = c // 2, c % 2
        cols = slice(g * F, (g + 1) * F)
        in_maps.append({
            "qT": np.ascontiguousarray(q[b].T).astype(bf),
            "kT": np.ascontiguousarray(k[b].T).astype(bf),
            "vT": np.ascontiguousarray(v[b].T).astype(bf),
            "wqT": np.ascontiguousarray(Wq[cols, :].T).astype(bf),
            "wkT": np.ascontiguousarray(Wk[cols, :].T).astype(bf),
            "wvT": np.ascontiguousarray(Wv[cols, :].T).astype(bf),
            "woT": np.ascontiguousarray(Wo[:, cols].T).astype(bf),
            "bqs": np.ascontiguousarray(bq[cols]),
            "bks": np.ascontiguousarray(bk[cols]),
            "bvs": np.ascontiguousarray(bv[cols]),
            # bo applied once per batch (head-group 0 only)
            "bos": np.ascontiguousarray(bo if g == 0 else np.zeros_like(bo)),
        })

    _CACHE["in_maps"] = in_maps
    trace = bool(int(os.environ.get("KERNEL_TRACE", "0")))
    res = run_bass_kernel_spmd(
        nc, in_maps, core_ids=list(range(NCORES)), trace=trace
    )
    if trace and res.exec_time_ns is not None:
        print(f"HW exec time: {res.exec_time_ns} ns")
    outs = [r["outT"] for r in res.results]
    out = np.empty((B, T, D), np.float32)
    for b in range(B):
        out[b] = (outs[2 * b] + outs[2 * b + 1]).T
    return out
